# revision 3
# baseline (speedup 1.0000x reference)
"""Causal self-attention (B=2, T=2048, C=2048, H=16) on 8 TRN2 NeuronCores.

Sharding: data-parallel over batch (2) x tensor-parallel over heads (4 heads
per core). Each core computes, for its batch element b and head group g:
  QKV projection for its heads' columns, causal attention for its 4 heads,
  and a partial output projection (row-sharded W_proj). The host sums the
  4 partial projections per batch element.

Device layouts (per core, fp16 compute / fp32 PSUM accumulation):
  xT   [C, T]      x_b transposed (host-side transpose)
  wqk  [C, 1024]   [Wq_h0..h3 | Wk_h0..h3], 128 cols per head
  wv   [C, 512]    Wv_h0..h3
  wp   [512, C]    W_proj rows for this head group
  out  [T, C] fp16 partial projection output

Attention per (head, 512-wide q-chunk), causality via loop bounds + 4
precomputed diagonal masks. All AV matmuls run in the transposed
orientation so the moving dim is the 512-wide q axis (not the 129-wide
head dim), keeping every PE instruction >= ~256 moving columns (the
LDWEIGHTS shadow):
  S^T[kt, q] = K_kt^T.T @ Q^T              (PE, one matmul per key tile kt)
  P^T = exp(scale * S^T)                   (ACT, PSUM->SBUF fp16)
  acc += P^T                               (DVE; per-k-row partial denom sums)
  yT[d, q] += V_kt[k, d].T @ P^T[k, q]     (PE, accumulated over kt in PSUM;
                                            output is already transposed for
                                            the projection - no PE transposes)
  den[1, q] = ones[k].T @ acc              (PE; partition reduction)
  recip = 1/den; bcast via ones[1,128].T @ recip   (DVE + PE)
  yt[d, q] = yT * recip_bcast              (DVE; fused normalize + evac)
Phase-1 chunk tj+1 is interleaved after attention chunk tj, and late
projection tiles after attention j=3, so the exp/normalize streams always
have PE work to hide behind.
"""

import os

import numpy as np

N_HEAD = 16
N_EMBD = 2048
B = 2
T = 2048
C = N_EMBD
D = C // N_HEAD  # 128
HPC = N_HEAD // 4  # heads per core = 4
N_CORES = 8
CT = C // 128  # 16 contraction tiles
TT = T // 128  # 16 t tiles
NCH = T // 512  # 4 chunks of 512

LAST_EXEC_NS = None

_CACHE = {}


def _build_nc():
    import concourse.bass as bass  # noqa: F401
    import concourse.tile as tile
    from concourse import bacc, mybir

    F32 = mybir.dt.float32
    F16 = mybir.dt.float16
    Exp = mybir.ActivationFunctionType.Exp
    Copy = mybir.ActivationFunctionType.Copy
    SCALE = 1.0 / float(np.sqrt(D))

    nc = bacc.Bacc("TRN2", target_bir_lowering=False, num_devices=N_CORES)

    xT_d = nc.dram_tensor("xT", [C, T], F16, kind="ExternalInput")
    wqk_d = nc.dram_tensor("wqk", [C, 8 * 128], F16, kind="ExternalInput")
    wv_d = nc.dram_tensor("wv", [C, 4 * 128], F16, kind="ExternalInput")
    wp_d = nc.dram_tensor("wp", [4 * 128, C], F16, kind="ExternalInput")
    out_d = nc.dram_tensor("out_part", [T, C], F16, kind="ExternalOutput")

    # Constants baked into the NEFF: diagonal causal masks.
    kk = np.arange(128)[:, None]
    qq = np.arange(512)[None, :]
    masks = np.stack(
        [(qq >= (128 * i + kk)).astype(np.float16) for i in range(4)]
    )  # [4, 128, 512]
    masks_d = nc.inline_tensor(np.ascontiguousarray(masks), name="diagmasks")

    with tile.TileContext(nc) as tc:
        with (
            tc.tile_pool(name="singles", bufs=1) as singles,
            tc.tile_pool(name="xtp", bufs=32) as xtp,
            tc.tile_pool(name="ptp", bufs=6) as ptp,
            tc.tile_pool(name="accp", bufs=2) as accp,
            tc.tile_pool(name="rowp", bufs=2) as rowp,
            tc.tile_pool(name="rbcp", bufs=2) as rbcp,
            tc.tile_pool(name="ost", bufs=6) as ostp,
            tc.tile_pool(name="ps", bufs=4, space="PSUM") as ps,
            tc.tile_pool(name="yps", bufs=2, space="PSUM") as yps,
            tc.tile_pool(name="denp", bufs=1, space="PSUM") as denp,
            tc.tile_pool(name="bcp", bufs=1, space="PSUM") as bcp,
        ):
            # x chunk 0 first (smallest first-matmul dependency), then wqk
            # column-sliced so the ct=0 accumulation chain can start after
            # ~1/8th of the weight bytes have landed.
            wqk_t = []
            xt0 = []
            for c in range(CT):
                xc = xtp.tile([128, 512], F16, tag="xt", name=f"xt0_{c}")
                nc.sync.dma_start(out=xc, in_=xT_d[c * 128 : (c + 1) * 128, 0:512])
                xt0.append(xc)
                wqk_t.append(singles.tile([128, 8 * 128], F16, name=f"wqkc{c}"))
            for ct in range(8):
                for c in range(CT):
                    nc.sync.dma_start(
                        out=wqk_t[c][:, ct * 128 : (ct + 1) * 128],
                        in_=wqk_d[c * 128 : (c + 1) * 128, ct * 128 : (ct + 1) * 128],
                    )
            wv_t = []

            # qkt: [d, coltile, t]; coltiles 0..3 = Q heads, 4..7 = K heads
            qkt_sb = singles.tile([128, 8, T], F16)
            # v: [kt-tile, head, d] - lhsT for the transposed AV matmuls
            vv_sb = singles.tile([128, TT, HPC, 128], F16)
            # y transposed: [d, head, t]
            yt_sb = singles.tile([128, HPC, T], F16)
            ones_k = singles.tile([128, 1], F16, name="ones_k")
            nc.vector.memset(ones_k, 1.0)
            ones_p = singles.tile([1, 128], F16, name="ones_p")
            nc.vector.memset(ones_p, 1.0)
            wp_sb = None
            mask_sb = None

            def phase1(tj):
                nonlocal wp_sb, mask_sb
                if tj == 0:
                    xt = xt0
                else:
                    xt = []
                    for c in range(CT):
                        xc = xtp.tile([128, 512], F16, tag="xt", name=f"xt{tj}_{c}")
                        nc.sync.dma_start(
                            out=xc,
                            in_=xT_d[
                                c * 128 : (c + 1) * 128, tj * 512 : (tj + 1) * 512
                            ],
                        )
                        xt.append(xc)
                for ct in range(8):
                    pq = ps.tile([128, 512], F32, tag="ps", name=f"pq{tj}_{ct}")
                    for c in range(CT):
                        nc.tensor.matmul(
                            pq,
                            wqk_t[c][:, ct * 128 : (ct + 1) * 128],
                            xt[c],
                            start=(c == 0),
                            stop=(c == CT - 1),
                        )
                    nc.scalar.activation(
                        out=qkt_sb[:, ct, tj * 512 : (tj + 1) * 512],
                        in_=pq,
                        func=Copy,
                    )
                if tj == 0:
                    # wv is first needed here; its DMA trails wqk/xt0.
                    for c in range(CT):
                        w = singles.tile([128, 512], F16, name=f"wvc{c}")
                        nc.sync.dma_start(out=w, in_=wv_d[c * 128 : (c + 1) * 128, :])
                        wv_t.append(w)
                for tt in range(4):
                    kt = tj * 4 + tt
                    pv = ps.tile([128, 512], F32, tag="ps", name=f"pv{kt}")
                    for c in range(CT):
                        nc.tensor.matmul(
                            pv,
                            xt[c][:, tt * 128 : (tt + 1) * 128],
                            wv_t[c],
                            start=(c == 0),
                            stop=(c == CT - 1),
                        )
                    nc.scalar.activation(
                        out=vv_sb[:, kt, :, :],
                        in_=pv.rearrange("p (h d) -> p h d", h=HPC),
                        func=Copy,
                    )
                if tj == 0:
                    # First needed by attention / projection; loaded early.
                    mask_sb = singles.tile([128, 4, 512], F16, name="mask_sb")
                    nc.sync.dma_start(
                        out=mask_sb, in_=masks_d[:, :, :].rearrange("a p n -> p a n")
                    )
                    wp_sb = singles.tile([128, HPC, C], F16, name="wp_sb")
                    nc.sync.dma_start(
                        out=wp_sb,
                        in_=wp_d[:, :].rearrange("(a p) n -> p a n", p=128),
                    )

            def attn_head(j, h):
                q0 = j * 512
                acc = accp.tile([128, 512], F16, tag="acc", name=f"acc{h}_{j}")
                yT = yps.tile([128, 512], F32, tag="y", name=f"yT{h}_{j}")
                nkt = 4 * j + 4
                for kt in range(nkt):
                    di = kt - 4 * j
                    lo = 128 * di if di > 0 else 0
                    ss = ps.tile([128, 512], F32, tag="ps", name=f"ss{h}{j}{kt}")
                    nc.tensor.matmul(
                        ss[:, lo:],
                        qkt_sb[:, 4 + h, kt * 128 : (kt + 1) * 128],
                        qkt_sb[:, h, q0 + lo : q0 + 512],
                        start=True,
                        stop=True,
                    )
                    pt = ptp.tile([128, 512], F16, tag="pt", name=f"pt{h}{j}{kt}")
                    nc.scalar.activation(
                        out=pt[:, lo:], in_=ss[:, lo:], func=Exp, scale=SCALE
                    )
                    if di >= 0:
                        nc.vector.tensor_mul(
                            pt[:, lo : lo + 128],
                            pt[:, lo : lo + 128],
                            mask_sb[:, di, lo : lo + 128],
                        )
                    if kt == 0:
                        nc.vector.tensor_copy(out=acc, in_=pt)
                    else:
                        nc.vector.tensor_add(
                            acc[:, lo:], acc[:, lo:], pt[:, lo:]
                        )
                    nc.tensor.matmul(
                        yT[:, lo:],
                        vv_sb[:, kt, h, :],
                        pt[:, lo:],
                        start=(kt == 0),
                        stop=(kt == nkt - 1),
                        skip_group_check=True,
                    )
                # Softmax denominator: partition-reduce acc on the PE, then
                # reciprocal + partition-broadcast, then fused normalize+evac.
                den = denp.tile([1, 512], F32, tag="den", name=f"den{h}_{j}")
                nc.tensor.matmul(den, ones_k, acc, start=True, stop=True)
                rrow = rowp.tile([1, 512], F16, tag="rr", name=f"rr{h}_{j}")
                with nc.allow_low_precision(reason="softmax denom reciprocal"):
                    nc.vector.reciprocal(rrow, den)
                bc = bcp.tile([128, 512], F32, tag="bc", name=f"bc{h}_{j}")
                nc.tensor.matmul(bc, ones_p, rrow, start=True, stop=True)
                rbc = rbcp.tile([128, 512], F16, tag="rbc", name=f"rbc{h}_{j}")
                nc.vector.tensor_copy(out=rbc, in_=bc)
                nc.vector.tensor_mul(yt_sb[:, h, q0 : q0 + 512], yT, rbc)

            def proj_tt(tt):
                for cc in range(4):
                    po = ps.tile([128, 512], F32, tag="ps", name=f"po{tt}_{cc}")
                    for hd in range(HPC):
                        nc.tensor.matmul(
                            po,
                            yt_sb[:, hd, tt * 128 : (tt + 1) * 128],
                            wp_sb[:, hd, cc * 512 : (cc + 1) * 512],
                            start=(hd == 0),
                            stop=(hd == HPC - 1),
                        )
                    oc = ostp.tile([128, 512], F16, tag="ot", name=f"ot{tt}_{cc}")
                    if cc % 2 == 0:
                        nc.vector.tensor_copy(out=oc, in_=po)
                    else:
                        nc.scalar.activation(out=oc, in_=po, func=Copy)
                    nc.sync.dma_start(
                        out=out_d[tt * 128 : (tt + 1) * 128, cc * 512 : (cc + 1) * 512],
                        in_=oc,
                    )

            # Phase-1 chunk tj feeds attention chunk j=tj (keys/values up to
            # tile 4*tj+3), so each attention chunk runs right after its
            # phase-1 chunk, hiding exp/normalize behind projection matmuls.
            for tj in range(NCH):
                phase1(tj)
                if tj < 3:
                    for h in range(HPC):
                        attn_head(tj, h)
            for h in range(HPC):
                attn_head(3, h)
                proj_tt(h)
            for tt in range(4, TT):
                proj_tt(tt)

    nc.compile()
    return nc


def _get_nc():
    if "nc" not in _CACHE:
        _CACHE["nc"] = _build_nc()
    return _CACHE["nc"]


def kernel(x, W_attn, W_proj):
    global LAST_EXEC_NS
    from concourse.bass_utils import run_bass_kernel_spmd

    x = np.asarray(x)
    W_attn = np.asarray(W_attn)
    W_proj = np.asarray(W_proj)

    in_maps = []
    for core in range(N_CORES):
        b, g = divmod(core, 4)
        heads = range(4 * g, 4 * g + 4)
        xT = np.ascontiguousarray(x[b].T).astype(np.float16)
        wqk = np.concatenate(
            [W_attn[:, h * D : (h + 1) * D] for h in heads]
            + [W_attn[:, C + h * D : C + (h + 1) * D] for h in heads],
            axis=1,
        ).astype(np.float16)
        wv = np.concatenate(
            [W_attn[:, 2 * C + h * D : 2 * C + (h + 1) * D] for h in heads], axis=1
        ).astype(np.float16)
        wp = W_proj[4 * g * D : 4 * (g + 1) * D, :].astype(np.float16)
        in_maps.append({"xT": xT, "wqk": wqk, "wv": wv, "wp": wp})

    nc = _get_nc()
    res = run_bass_kernel_spmd(
        nc,
        in_maps,
        list(range(N_CORES)),
        trace=bool(os.environ.get("KERNEL_TRACE")),
    )
    LAST_EXEC_NS = res.exec_time_ns

    out = np.zeros((B, T, C), dtype=np.float32)
    for core in range(N_CORES):
        b = core // 4
        out[b] += res.results[core]["out_part"]
    return out


# revision 4
# speedup vs baseline: 1.0518x; 1.0518x over previous
"""Causal self-attention (B=2, T=2048, C=2048, H=16) on 8 TRN2 NeuronCores.

Sharding: data-parallel over batch (2) x tensor-parallel over heads (4 heads
per core). Each core computes, for its batch element b and head group g:
  QKV projection for its heads' columns, causal attention for its 4 heads,
  and a partial output projection (row-sharded W_proj). The host sums the
  4 partial projections per batch element.

Device layouts (per core, fp16 compute / fp32 PSUM accumulation):
  xT   [C, T]      x_b transposed (host-side transpose)
  wqk  [C, 1024]   [Wq_h0..h3 | Wk_h0..h3], 128 cols per head
  wv   [C, 512]    Wv_h0..h3
  wp   [512, C]    W_proj rows for this head group
  out  [T, C] fp16 partial projection output

Attention per (head, 512-wide q-chunk), causality via loop bounds + 4
precomputed diagonal masks. AV runs in the transposed orientation so every
PE instruction is ~512 moving columns (LDWEIGHTS stays hidden) and the
projection consumes yT directly (no PE transposes):
  S^T[kt, q] = K_kt^T.T @ Q^T              (PE)
  P^T = exp(scale * S^T)                   (ACT, PSUM->SBUF fp16)
  acc += P^T                               (DVE, denominator partial sums)
  yT[d, q] += V_kt[k, d].T @ P^T[k, q]     (PE, PSUM-accumulated over kt)
  den_bc = ones[128,128].T @ acc           (PE; partition-sum broadcast to
                                            all 128 partitions in one matmul)
  rbc = 1/den_bc                           (DVE)
  yt[d, q] = yT * rbc                      (DVE, fused normalize + evac)

Scheduling: the attention kt-chain is exp(ACT)-bound (~580ns/kt vs ~430ns of
PE work), so attention chunk j is interleaved INTO phase-1 chunk j+1 at
quarter-chain granularity (phase 1 is pure PE; its windows absorb the ACT/DVE
streams), attention j=3 is padded with projection matmul fillers, and each
head's denominator reduction is flushed one head late so the PE never waits
on the DVE accumulation chain.
"""

import os

import numpy as np

N_HEAD = 16
N_EMBD = 2048
B = 2
T = 2048
C = N_EMBD
D = C // N_HEAD  # 128
HPC = N_HEAD // 4  # heads per core = 4
N_CORES = 8
CT = C // 128  # 16 contraction tiles
TT = T // 128  # 16 t tiles
NCH = T // 512  # 4 chunks of 512

LAST_EXEC_NS = None

_CACHE = {}


def _build_nc():
    import concourse.bass as bass  # noqa: F401
    import concourse.tile as tile
    from concourse import bacc, mybir

    F32 = mybir.dt.float32
    F16 = mybir.dt.float16
    Exp = mybir.ActivationFunctionType.Exp
    Copy = mybir.ActivationFunctionType.Copy
    SCALE = 1.0 / float(np.sqrt(D))

    nc = bacc.Bacc("TRN2", target_bir_lowering=False, num_devices=N_CORES)

    xT_d = nc.dram_tensor("xT", [C, T], F16, kind="ExternalInput")
    wqk_d = nc.dram_tensor("wqk", [C, 8 * 128], F16, kind="ExternalInput")
    wv_d = nc.dram_tensor("wv", [C, 4 * 128], F16, kind="ExternalInput")
    wp_d = nc.dram_tensor("wp", [4 * 128, C], F16, kind="ExternalInput")
    out_d = nc.dram_tensor("out_part", [T, C], F16, kind="ExternalOutput")

    # Constants baked into the NEFF: diagonal causal masks.
    kk = np.arange(128)[:, None]
    qq = np.arange(512)[None, :]
    masks = np.stack(
        [(qq >= (128 * i + kk)).astype(np.float16) for i in range(4)]
    )  # [4, 128, 512]
    masks_d = nc.inline_tensor(np.ascontiguousarray(masks), name="diagmasks")

    with tile.TileContext(nc) as tc:
        with (
            tc.tile_pool(name="singles", bufs=1) as singles,
            tc.tile_pool(name="xtp", bufs=32) as xtp,
            tc.tile_pool(name="ptp", bufs=6) as ptp,
            tc.tile_pool(name="accp", bufs=3) as accp,
            tc.tile_pool(name="rbcp", bufs=2) as rbcp,
            tc.tile_pool(name="ost", bufs=6) as ostp,
            tc.tile_pool(name="ps", bufs=4, space="PSUM") as ps,
            tc.tile_pool(name="yps", bufs=3, space="PSUM") as yps,
            tc.tile_pool(name="bcp", bufs=1, space="PSUM") as bcp,
        ):
            # x chunk 0 first (smallest first-matmul dependency), then wqk
            # column-sliced so the ct=0 accumulation chain can start after
            # ~1/8th of the weight bytes have landed.
            wqk_t = []
            xt_tiles = {}

            def issue_x_dma(tj):
                xs = []
                for c in range(CT):
                    xc = xtp.tile([128, 512], F16, tag="xt", name=f"xt{tj}_{c}")
                    nc.sync.dma_start(
                        out=xc,
                        in_=xT_d[c * 128 : (c + 1) * 128, tj * 512 : (tj + 1) * 512],
                    )
                    xs.append(xc)
                xt_tiles[tj] = xs

            issue_x_dma(0)
            for c in range(CT):
                wqk_t.append(singles.tile([128, 8 * 128], F16, name=f"wqkc{c}"))
            for ct in range(8):
                for c in range(CT):
                    nc.sync.dma_start(
                        out=wqk_t[c][:, ct * 128 : (ct + 1) * 128],
                        in_=wqk_d[c * 128 : (c + 1) * 128, ct * 128 : (ct + 1) * 128],
                    )
            wv_t = []

            # qkt: [d, coltile, t]; coltiles 0..3 = Q heads, 4..7 = K heads
            qkt_sb = singles.tile([128, 8, T], F16)
            # v: [kt-tile, head, d] - lhsT for the transposed AV matmuls
            vv_sb = singles.tile([128, TT, HPC, 128], F16)
            # y transposed: [d, head, t]
            yt_sb = singles.tile([128, HPC, T], F16)
            ones_pp = singles.tile([128, 128], F16, name="ones_pp")
            nc.vector.memset(ones_pp, 1.0)
            wp_sb = None
            mask_sb = None

            # ---------------- attention unit stream ----------------
            def make_att_units(j):
                """List of closures; each emits one attention kt-step for
                chunk j. Denominator flush for head h is embedded two steps
                into head h+1 (the final head's flush is appended last)."""
                q0 = j * 512
                nkt = 4 * j + 4
                units = []

                def make_den(st):
                    def den():
                        bc = bcp.tile([128, 512], F32, tag="bc")
                        nc.tensor.matmul(bc, ones_pp, st["acc"], start=True, stop=True)
                        rbc = rbcp.tile([128, 512], F16, tag="rbc")
                        with nc.allow_low_precision(reason="softmax denom recip"):
                            nc.vector.reciprocal(rbc, bc)
                        nc.vector.tensor_mul(
                            yt_sb[:, st["h"], q0 : q0 + 512], st["yT"], rbc
                        )

                    return den

                prev_den = None
                for h in range(HPC):
                    st = {"h": h}
                    head_units = []
                    for kt in range(nkt):
                        def unit(kt=kt, h=h, st=st):
                            if kt == 0:
                                st["acc"] = accp.tile(
                                    [128, 512], F16, tag="acc", name=f"acc{h}_{j}"
                                )
                                st["yT"] = yps.tile(
                                    [128, 512], F32, tag="y", name=f"yT{h}_{j}"
                                )
                            di = kt - 4 * j
                            lo = 128 * di if di > 0 else 0
                            ss = ps.tile([128, 512], F32, tag="ps", name=f"ss{h}{j}{kt}")
                            nc.tensor.matmul(
                                ss[:, lo:],
                                qkt_sb[:, 4 + h, kt * 128 : (kt + 1) * 128],
                                qkt_sb[:, h, q0 + lo : q0 + 512],
                                start=True,
                                stop=True,
                                skip_group_check=True,
                            )
                            pt = ptp.tile(
                                [128, 512], F16, tag="pt", name=f"pt{h}{j}{kt}"
                            )
                            nc.scalar.activation(
                                out=pt[:, lo:], in_=ss[:, lo:], func=Exp, scale=SCALE
                            )
                            if di >= 0:
                                nc.vector.tensor_mul(
                                    pt[:, lo : lo + 128],
                                    pt[:, lo : lo + 128],
                                    mask_sb[:, di, lo : lo + 128],
                                )
                            if kt == 0:
                                nc.vector.tensor_copy(out=st["acc"], in_=pt)
                            else:
                                nc.vector.tensor_add(
                                    st["acc"][:, lo:], st["acc"][:, lo:], pt[:, lo:]
                                )
                            nc.tensor.matmul(
                                st["yT"][:, lo:],
                                vv_sb[:, kt, h, :],
                                pt[:, lo:],
                                start=(kt == 0),
                                stop=(kt == nkt - 1),
                                skip_group_check=True,
                            )

                        head_units.append(unit)
                    if prev_den is not None:
                        head_units.insert(min(2, len(head_units)), prev_den)
                    prev_den = make_den(st)
                    units.extend(head_units)
                units.append(prev_den)
                return units

            # ---------------- phase 1 with interleaved attention ----------------
            def phase1(tj, units):
                nonlocal wp_sb, mask_sb
                xt = xt_tiles[tj]
                if tj + 1 < NCH:
                    issue_x_dma(tj + 1)
                n_points = 12 * 4  # insertion points: 4 per chain, 12 chains
                point = 0
                done = 0

                def pull():
                    nonlocal point, done
                    point += 1
                    target = (len(units) * point) // n_points
                    while done < target:
                        units[done]()
                        done += 1

                for ct in range(8):
                    pq = ps.tile([128, 512], F32, tag="ps", name=f"pq{tj}_{ct}")
                    for c in range(CT):
                        nc.tensor.matmul(
                            pq,
                            wqk_t[c][:, ct * 128 : (ct + 1) * 128],
                            xt[c],
                            start=(c == 0),
                            stop=(c == CT - 1),
                            skip_group_check=True,
                        )
                        if c % 4 == 3:
                            pull()
                    nc.scalar.activation(
                        out=qkt_sb[:, ct, tj * 512 : (tj + 1) * 512],
                        in_=pq,
                        func=Copy,
                    )
                if tj == 0:
                    # wv is first needed here; its DMA trails wqk/xt0.
                    for c in range(CT):
                        w = singles.tile([128, 512], F16, name=f"wvc{c}")
                        nc.sync.dma_start(out=w, in_=wv_d[c * 128 : (c + 1) * 128, :])
                        wv_t.append(w)
                for tt in range(4):
                    kt = tj * 4 + tt
                    pv = ps.tile([128, 512], F32, tag="ps", name=f"pv{kt}")
                    for c in range(CT):
                        nc.tensor.matmul(
                            pv,
                            xt[c][:, tt * 128 : (tt + 1) * 128],
                            wv_t[c],
                            start=(c == 0),
                            stop=(c == CT - 1),
                            skip_group_check=True,
                        )
                        if c % 4 == 3:
                            pull()
                    nc.scalar.activation(
                        out=vv_sb[:, kt, :, :],
                        in_=pv.rearrange("p (h d) -> p h d", h=HPC),
                        func=Copy,
                    )
                if tj == 0:
                    # First needed by attention / projection; loaded early.
                    mask_sb = singles.tile([128, 4, 512], F16, name="mask_sb")
                    nc.sync.dma_start(
                        out=mask_sb, in_=masks_d[:, :, :].rearrange("a p n -> p a n")
                    )
                    wp_sb = singles.tile([128, HPC, C], F16, name="wp_sb")
                    nc.sync.dma_start(
                        out=wp_sb,
                        in_=wp_d[:, :].rearrange("(a p) n -> p a n", p=128),
                    )
                while done < len(units):
                    units[done]()
                    done += 1

            # ---------------- projection unit stream ----------------
            def make_proj_units(tts, evac_dve):
                """Per-matmul-granularity projection closures for t-tiles tts.
                The 4th matmul of each (tt, cc) block bundles evac + DMA out."""
                units = []
                for tt in tts:
                    for cc in range(4):
                        po_box = {}

                        def mk(hd, tt=tt, cc=cc, po_box=po_box, first=None):
                            def u():
                                if hd == 0:
                                    po_box["po"] = ps.tile(
                                        [128, 512], F32, tag="ps", name=f"po{tt}_{cc}"
                                    )
                                nc.tensor.matmul(
                                    po_box["po"],
                                    yt_sb[:, hd, tt * 128 : (tt + 1) * 128],
                                    wp_sb[:, hd, cc * 512 : (cc + 1) * 512],
                                    start=(hd == 0),
                                    stop=(hd == HPC - 1),
                                    skip_group_check=True,
                                )
                                if hd == HPC - 1:
                                    po = po_box["po"]
                                    oc = ostp.tile(
                                        [128, 512], F16, tag="ot", name=f"ot{tt}_{cc}"
                                    )
                                    if evac_dve or cc % 2 == 0:
                                        nc.vector.tensor_copy(out=oc, in_=po)
                                    else:
                                        nc.scalar.activation(out=oc, in_=po, func=Copy)
                                    for half in range(2):
                                        nc.sync.dma_start(
                                            out=out_d[
                                                tt * 128 + half * 64 : tt * 128
                                                + (half + 1) * 64,
                                                cc * 512 : (cc + 1) * 512,
                                            ],
                                            in_=oc[half * 64 : (half + 1) * 64, :],
                                        )

                            return u

                        for hd in range(HPC):
                            units.append(mk(hd))
                return units

            # ---------------- schedule ----------------
            phase1(0, [])
            phase1(1, make_att_units(0))
            phase1(2, make_att_units(1))
            phase1(3, make_att_units(2))

            # attention chunk 3: exp-bound; pad with projection fillers
            # (chunks 0-2 only; chunk-3 projections need chunk-3 normalize).
            att3 = make_att_units(3)
            fillers = make_proj_units(range(12), evac_dve=True)
            fi = 0
            for ui, u in enumerate(att3):
                u()
                # ~2 filler matmuls per 3 attention steps fills the PE's
                # exp-wait; start after the first den flush has landed.
                if ui >= 6:
                    target = min((ui - 6) * 2 // 3, len(fillers))
                    while fi < target:
                        fillers[fi]()
                        fi += 1
            while fi < len(fillers):
                fillers[fi]()
                fi += 1
            for u in make_proj_units(range(12, TT), evac_dve=False):
                u()

    nc.compile()
    return nc


def _get_nc():
    if "nc" not in _CACHE:
        _CACHE["nc"] = _build_nc()
    return _CACHE["nc"]


def kernel(x, W_attn, W_proj):
    global LAST_EXEC_NS
    from concourse.bass_utils import run_bass_kernel_spmd

    x = np.asarray(x)
    W_attn = np.asarray(W_attn)
    W_proj = np.asarray(W_proj)

    in_maps = []
    for core in range(N_CORES):
        b, g = divmod(core, 4)
        heads = range(4 * g, 4 * g + 4)
        xT = np.ascontiguousarray(x[b].T).astype(np.float16)
        wqk = np.concatenate(
            [W_attn[:, h * D : (h + 1) * D] for h in heads]
            + [W_attn[:, C + h * D : C + (h + 1) * D] for h in heads],
            axis=1,
        ).astype(np.float16)
        wv = np.concatenate(
            [W_attn[:, 2 * C + h * D : 2 * C + (h + 1) * D] for h in heads], axis=1
        ).astype(np.float16)
        wp = W_proj[4 * g * D : 4 * (g + 1) * D, :].astype(np.float16)
        in_maps.append({"xT": xT, "wqk": wqk, "wv": wv, "wp": wp})

    nc = _get_nc()
    res = run_bass_kernel_spmd(
        nc,
        in_maps,
        list(range(N_CORES)),
        trace=bool(os.environ.get("KERNEL_TRACE")),
    )
    LAST_EXEC_NS = res.exec_time_ns

    out = np.zeros((B, T, C), dtype=np.float32)
    for core in range(N_CORES):
        b = core // 4
        out[b] += res.results[core]["out_part"]
    return out


# revision 9
# speedup vs baseline: 1.3194x; 1.2544x over previous
"""Causal self-attention (B=2, T=2048, C=2048, H=16) on 8 TRN2 NeuronCores.

Sharding: data-parallel over batch (2) x tensor-parallel over heads (4 heads
per core). Each core computes, for its batch element b and head group g:
  QKV projection for its heads' columns, causal attention for its 4 heads,
  and a partial output projection (row-sharded W_proj). The host sums the
  4 partial projections per batch element.

Device layouts (per core, fp16 compute / fp32 PSUM accumulation):
  xT   [C, T]      x_b transposed (host-side transpose)
  wqk  [C, 1024]   [Wq_h0..h3 | Wk_h0..h3], 128 cols per head
  wv   [C, 512]    Wv_h0..h3
  wp   [512, C]    W_proj rows for this head group
  out  [T, C] fp32 partial projection output

Attention per (head, 512-wide q-chunk), exploiting causality via loop
bounds and 4 precomputed diagonal masks:
  S^T[kt, q] = K_kt^T.T @ Q^T            (PE, one matmul per key tile kt)
  P^T = exp(scale * S^T)                 (ACT, PSUM->SBUF fp16)
  Y[q, d+1] += P^T.T @ [V | ones]        (PE, accumulated over kt in PSUM;
                                          the ones column yields the softmax
                                          denominator for free)
  y = Y[:, :d] * (1 / Y[:, d])           (DVE, per-partition scalar)
  y^T via PE transpose -> yt[d, head, t] (layout the projection consumes)
The N=129 AV matmuls trade some PE efficiency for a fully local pipeline
(no cross-engine reduction chains); measured, this keeps the PE ~98% busy
between phases and the HAM clock-gate warm.
"""

import os

import numpy as np

N_HEAD = 16
N_EMBD = 2048
B = 2
T = 2048
C = N_EMBD
D = C // N_HEAD  # 128
HPC = N_HEAD // 4  # heads per core = 4
N_CORES = 8
CT = C // 128  # 16 contraction tiles
TT = T // 128  # 16 t tiles
NCH = T // 512  # 4 chunks of 512

LAST_EXEC_NS = None

_CACHE = {}


def _build_nc():
    import concourse.bass as bass  # noqa: F401
    import concourse.tile as tile
    from concourse import bacc, mybir

    F32 = mybir.dt.float32
    F16 = mybir.dt.float16
    Exp = mybir.ActivationFunctionType.Exp
    Copy = mybir.ActivationFunctionType.Copy
    SCALE = 1.0 / float(np.sqrt(D))

    nc = bacc.Bacc("TRN2", target_bir_lowering=False, num_devices=N_CORES)

    xT_d = nc.dram_tensor("xT", [C, T], F16, kind="ExternalInput")
    wqk_d = nc.dram_tensor("wqk", [C, 8 * 128], F16, kind="ExternalInput")
    wv_d = nc.dram_tensor("wv", [C, 4 * 128], F16, kind="ExternalInput")
    wp_d = nc.dram_tensor("wp", [4 * 128, C], F16, kind="ExternalInput")
    out_d = nc.dram_tensor("out_part", [T, C], F32, kind="ExternalOutput")

    # Constants baked into the NEFF: diagonal causal masks and identity.
    kk = np.arange(128)[:, None]
    qq = np.arange(512)[None, :]
    masks = np.stack(
        [(qq >= (128 * i + kk)).astype(np.float16) for i in range(4)]
    )  # [4, 128, 512]
    masks_d = nc.inline_tensor(np.ascontiguousarray(masks), name="diagmasks")
    ident_d = nc.inline_tensor(np.eye(128, dtype=np.float16), name="ident128")

    with tile.TileContext(nc) as tc:
        with (
            tc.tile_pool(name="singles", bufs=1) as singles,
            tc.tile_pool(name="xtp", bufs=32) as xtp,
            tc.tile_pool(name="ptp", bufs=6) as ptp,
            tc.tile_pool(name="ysb", bufs=4) as ysbp,
            tc.tile_pool(name="rp", bufs=4) as rp,
            tc.tile_pool(name="ost", bufs=3) as ostp,
            tc.tile_pool(name="ps", bufs=4, space="PSUM") as ps,
            tc.tile_pool(name="yps", bufs=4, space="PSUM") as yps,
        ):
            # Per-c-tile weight loads, interleaved with the first x chunk, so
            # the first matmuls wait on ~512 KB, not the whole input set.
            wqk_t = []
            wv_t = []
            xt0 = []
            for c in range(CT):
                xc = xtp.tile([128, 512], F16, tag="xt", name=f"xt0_{c}")
                nc.sync.dma_start(out=xc, in_=xT_d[c * 128 : (c + 1) * 128, 0:512])
                xt0.append(xc)
                w = singles.tile([128, 8 * 128], F16, name=f"wqkc{c}")
                nc.sync.dma_start(out=w, in_=wqk_d[c * 128 : (c + 1) * 128, :])
                wqk_t.append(w)

            # qkt: [d, coltile, t]; coltiles 0..3 = Q heads, 4..7 = K heads
            qkt_sb = singles.tile([128, 8, T], F16)
            # v with a ones column per (kt, head): [kt-tile, head, 129]
            vv_sb = singles.tile([128, TT, HPC, 129], F16)
            # y transposed: [d, head, t]
            yt_sb = singles.tile([128, HPC, T], F16)
            wp_sb = None
            mask_sb = None
            ident_sb = None

            # ---- Phase 1: QKV projection ----
            for tj in range(NCH):
                if tj == 0:
                    xt = xt0
                else:
                    xt = []
                    for c in range(CT):
                        xc = xtp.tile([128, 512], F16, tag="xt", name=f"xt{tj}_{c}")
                        nc.sync.dma_start(
                            out=xc,
                            in_=xT_d[
                                c * 128 : (c + 1) * 128, tj * 512 : (tj + 1) * 512
                            ],
                        )
                        xt.append(xc)
                for ct in range(8):
                    pq = ps.tile([128, 512], F32, tag="ps", name=f"pq{tj}_{ct}")
                    for c in range(CT):
                        nc.tensor.matmul(
                            pq,
                            wqk_t[c][:, ct * 128 : (ct + 1) * 128],
                            xt[c],
                            start=(c == 0),
                            stop=(c == CT - 1),
                        )
                    nc.scalar.activation(
                        out=qkt_sb[:, ct, tj * 512 : (tj + 1) * 512],
                        in_=pq,
                        func=Copy,
                    )
                if tj == 0:
                    # wv is first needed here; its DMA trails wqk/xt0.
                    for c in range(CT):
                        w = singles.tile([128, 512], F16, name=f"wvc{c}")
                        nc.sync.dma_start(out=w, in_=wv_d[c * 128 : (c + 1) * 128, :])
                        wv_t.append(w)
                for tt in range(4):
                    kt = tj * 4 + tt
                    pv = ps.tile([128, 512], F32, tag="ps", name=f"pv{kt}")
                    for c in range(CT):
                        nc.tensor.matmul(
                            pv,
                            xt[c][:, tt * 128 : (tt + 1) * 128],
                            wv_t[c],
                            start=(c == 0),
                            stop=(c == CT - 1),
                        )
                    nc.scalar.activation(
                        out=vv_sb[:, kt, :, 0:128],
                        in_=pv.rearrange("p (h d) -> p h d", h=HPC),
                        func=Copy,
                    )
                    nc.vector.memset(vv_sb[:, kt, :, 128:129], 1.0)
                if tj == 0:
                    # First needed by attention; loaded during phase 1.
                    wp_sb = singles.tile([128, HPC, C], F16, name="wp_sb")
                    nc.sync.dma_start(
                        out=wp_sb,
                        in_=wp_d[:, :].rearrange("(a p) n -> p a n", p=128),
                    )
                    mask_sb = singles.tile([128, 4, 512], F16, name="mask_sb")
                    nc.sync.dma_start(
                        out=mask_sb, in_=masks_d[:, :, :].rearrange("a p n -> p a n")
                    )
                    ident_sb = singles.tile([128, 128], F16, name="ident_sb")
                    nc.sync.dma_start(out=ident_sb, in_=ident_d[:, :])

            # ---- Phases 2+3 interleaved per q-chunk ----
            for j in range(NCH):
                for h in range(HPC):
                    y_tiles = [
                        yps.tile([128, 129], F32, tag="y", name=f"ytile{h}_{j}_{qs}")
                        for qs in range(4)
                    ]
                    for kt in range(4 * j + 4):
                        di = kt - 4 * j
                        lo = 128 * di if di > 0 else 0
                        ss = ps.tile([128, 512], F32, tag="ps", name=f"ss{h}{j}{kt}")
                        nc.tensor.matmul(
                            ss[:, lo:],
                            qkt_sb[:, 4 + h, kt * 128 : (kt + 1) * 128],
                            qkt_sb[:, h, j * 512 + lo : (j + 1) * 512],
                            start=True,
                            stop=True,
                        )
                        pt = ptp.tile([128, 512], F16, tag="pt", name=f"pt{h}{j}{kt}")
                        nc.scalar.activation(
                            out=pt[:, lo:], in_=ss[:, lo:], func=Exp, scale=SCALE
                        )
                        if di >= 0:
                            nc.vector.tensor_mul(
                                pt[:, lo : lo + 128],
                                pt[:, lo : lo + 128],
                                mask_sb[:, di, lo : lo + 128],
                            )
                        for qs in range(max(0, di), 4):
                            nc.tensor.matmul(
                                y_tiles[qs],
                                pt[:, qs * 128 : (qs + 1) * 128],
                                vv_sb[:, kt, h, :],
                                start=(kt == 0),
                                stop=(kt == 4 * j + qs),
                            )
                    for qs in range(4):
                        yt = y_tiles[qs]
                        r = rp.tile([128, 1], F32, tag="r", name=f"r{h}{j}{qs}")
                        nc.vector.reciprocal(r, yt[:, 128:129])
                        y16 = ysbp.tile([128, 128], F16, tag="y16", name=f"y16_{qs}")
                        nc.vector.tensor_scalar_mul(y16, yt[:, 0:128], r)
                        ytp = yps.tile([128, 128], F16, tag="y", name=f"ytp{h}{j}{qs}")
                        nc.tensor.transpose(ytp, y16, ident_sb)
                        tglob = (j * 4 + qs) * 128
                        nc.scalar.activation(
                            out=yt_sb[:, h, tglob : tglob + 128], in_=ytp, func=Copy
                        )

                for tt in range(4 * j, 4 * j + 4):
                    ot = ostp.tile([128, C], F32, tag="ot", name=f"ot{tt}")
                    for cc in range(4):
                        po = ps.tile([128, 512], F32, tag="ps", name=f"po{tt}_{cc}")
                        for hd in range(HPC):
                            nc.tensor.matmul(
                                po,
                                yt_sb[:, hd, tt * 128 : (tt + 1) * 128],
                                wp_sb[:, hd, cc * 512 : (cc + 1) * 512],
                                start=(hd == 0),
                                stop=(hd == HPC - 1),
                            )
                        if cc % 2 == 0:
                            nc.vector.tensor_copy(
                                out=ot[:, cc * 512 : (cc + 1) * 512], in_=po
                            )
                        else:
                            nc.scalar.activation(
                                out=ot[:, cc * 512 : (cc + 1) * 512], in_=po, func=Copy
                            )
                    nc.sync.dma_start(out=out_d[tt * 128 : (tt + 1) * 128, :], in_=ot)

    nc.compile()
    return nc


def _get_nc():
    if "nc" not in _CACHE:
        _CACHE["nc"] = _build_nc()
    return _CACHE["nc"]


def kernel(x, W_attn, W_proj):
    global LAST_EXEC_NS
    from concourse.bass_utils import run_bass_kernel_spmd

    x = np.asarray(x)
    W_attn = np.asarray(W_attn)
    W_proj = np.asarray(W_proj)

    in_maps = []
    for core in range(N_CORES):
        b, g = divmod(core, 4)
        heads = range(4 * g, 4 * g + 4)
        xT = np.ascontiguousarray(x[b].T).astype(np.float16)
        wqk = np.concatenate(
            [W_attn[:, h * D : (h + 1) * D] for h in heads]
            + [W_attn[:, C + h * D : C + (h + 1) * D] for h in heads],
            axis=1,
        ).astype(np.float16)
        wv = np.concatenate(
            [W_attn[:, 2 * C + h * D : 2 * C + (h + 1) * D] for h in heads], axis=1
        ).astype(np.float16)
        wp = W_proj[4 * g * D : 4 * (g + 1) * D, :].astype(np.float16)
        in_maps.append({"xT": xT, "wqk": wqk, "wv": wv, "wp": wp})

    nc = _get_nc()
    res = run_bass_kernel_spmd(
        nc,
        in_maps,
        list(range(N_CORES)),
        trace=bool(os.environ.get("KERNEL_TRACE")),
    )
    LAST_EXEC_NS = res.exec_time_ns

    out = np.zeros((B, T, C), dtype=np.float32)
    for core in range(N_CORES):
        b = core // 4
        out[b] += res.results[core]["out_part"]
    return out
